# revision 8
# baseline (speedup 1.0000x reference)
"""Trainium2 Bass kernel for nn_CTRule (temporal KG scoring model).

Computes, for each of B=1024 queries (h, r, t):
  v = f(E0[h], E1[r], time tables, rule tables)   # [B, 128] elementwise algebra
  scores = v @ E0.T                               # [B, 40000]

Host-side table folding (index-independent, O(table-size) only):
  complex view u = u[:64] + i*u[64:].  The reference's rule/no-rule select
  collapses per relation r into three tables (hr = has_rules[r]):
    W0 = hr ? R - rule_S*|R|^2          : R
    W1 = hr ? conj(R)*(1 + rule_C)      : conj(R)
    W2 = hr ? 0                         : conj(R) + |R|^2
  so that  rel_ = W0[r] + CT*W1[r] + L*W2[r]   (complex products), and
    v = L*(rel_ + TM) + TE*conj(rel_ - TM)
  with TM = (E2 + E5[t//C])[t], TE = (E3 + E6[t//C])[t], CT = E4[t].

Distribution over 8 cores: 4-way model parallel over entities x 2-way data
parallel over the batch.  Core c = sb*4 + se handles batch rows
[sb*512, sb*512+512) (4 tiles of 128) x entity columns
[se*10000, se*10000+10000).  Per core:
  * 3 batched indirect gathers (SWDGE) pull W / L / T rows for all 4 tiles,
  * the bf16 elementwise head runs for tiles 0-1 on VectorE (2x_1P mode,
    two tiles per instruction) and tiles 2-3 on GpSimd in parallel,
  * vT transposes on TensorE (copied out of PSUM by ScalarE), then a
    500-column matmul stream against the bf16 E0T slice with PSUM->SBUF
    bf16 casts split 12:8 between ScalarE and VectorE,
  * [128, 2500] output blocks stream out on the sync-engine HWDGE ring.
No cross-core communication; the host reassembles the 8 blocks and casts f32.
"""

import numpy as np
import ml_dtypes

P = 128
B = 1024
RANK = 128
NENT = 40000
NREL = 230
NTIME = 365
CYCLE = 120
NCORES = 8
SE = 4                   # entity shards
SB = 2                   # batch shards
NCOL = NENT // SE        # entity columns per core = 10000
TILES = B // SB // P     # batch tiles per core = 4
CHUNK = 500              # matmul / PSUM chunk columns
OUTCH = 2500             # output DMA chunk columns
ECH = 2500               # E0T load chunk columns

W_W = 3 * RANK           # [W0 | W1 | W2] = 384
T_W = 3 * RANK           # [CT | TM | TE] = 384

# cast-engine schedule per tile (20 chunks): a=scalar(ACT), d=vector(DVE)
CAST_PAT = list("a" * 20)
for _k in (2, 5, 8, 11, 14, 17, 19):
    CAST_PAT[_k] = "d"
CAST_PAT = "".join(CAST_PAT)
# output DMA group boundaries (columns) per tile: small first group so the
# out stream starts early
OUT_EDGES = [0, 1000, 4000, 7000, 10000]

TRACE = False            # set by test harness for profiling runs
_CACHE = {}


def _build():
    import concourse.bass as bass
    import concourse.mybir as mybir
    import concourse.tile as tile
    from concourse import bacc
    from concourse.masks import make_identity

    dt = mybir.dt
    mult = mybir.AluOpType.mult
    add = mybir.AluOpType.add
    sub = mybir.AluOpType.subtract

    nc = bacc.Bacc("TRN2", target_bir_lowering=False, debug=False,
                   num_devices=NCORES)

    IDX = nc.dram_tensor("IDX", [P, 3, TILES], dt.int32, kind="ExternalInput").ap()
    E0B = nc.dram_tensor("E0B", [NENT, RANK], dt.bfloat16, kind="ExternalInput").ap()
    WCAT = nc.dram_tensor("WCAT", [NREL, W_W], dt.bfloat16, kind="ExternalInput").ap()
    TCAT = nc.dram_tensor("TCAT", [NTIME, T_W], dt.bfloat16, kind="ExternalInput").ap()
    E0T = nc.dram_tensor("E0T", [RANK, NCOL], dt.bfloat16, kind="ExternalInput").ap()
    OUT = nc.dram_tensor("OUT", [TILES * P, NCOL], dt.bfloat16,
                         kind="ExternalOutput").ap()

    with tile.TileContext(nc) as tc:
        with (
            tc.tile_pool(name="const", bufs=1) as constp,
            tc.tile_pool(name="gath", bufs=1) as gp,
            tc.tile_pool(name="ewa", bufs=1) as ewa,
            tc.tile_pool(name="ewb", bufs=1) as ewb,
            tc.tile_pool(name="vtp", bufs=2, space="PSUM") as vtp,
            tc.tile_pool(name="psm", bufs=6, space="PSUM") as psm,
            tc.tile_pool(name="osbp", bufs=2) as osbp,
        ):
            # ---- index load + E0T stream on the sync HWDGE ring (fast path,
            # no gpsimd involvement); OUT blocks join the same ring later.
            idxt = gp.tile([P, 3, TILES], dt.int32)
            nc.sync.dma_start(idxt[:], IDX[:])
            e0t = constp.tile([RANK, NCOL], dt.bfloat16)
            for c0 in range(0, NCOL, ECH):
                nc.sync.dma_start(e0t[:, c0:c0 + ECH], E0T[:, c0:c0 + ECH])

            # ---- per-tile indirect gathers on SWDGE (the HW consumes exactly
            # one offset per partition per DMA), in tile order so tile 0's
            # head starts as early as possible
            Lg = gp.tile([P, TILES, RANK], dt.bfloat16)
            Wg = gp.tile([P, TILES, W_W], dt.bfloat16)
            Tg = gp.tile([P, TILES, T_W], dt.bfloat16)
            for j in range(TILES):
                for dst, src, col in ((Wg, WCAT, 1), (Lg, E0B, 0),
                                      (Tg, TCAT, 2)):
                    nc.gpsimd.indirect_dma_start(
                        out=dst[:, j, :], out_offset=None, in_=src[:],
                        in_offset=bass.IndirectOffsetOnAxis(
                            ap=idxt[:, col, j:j + 1], axis=0))

            identb = constp.tile([P, P], dt.bfloat16)
            make_identity(nc, identb[:])

            # ---- elementwise head, all bf16, one tile per pass:
            # tiles 0-1 on VectorE (fast start), tiles 2-3 on GpSimd.
            def head(eng, pool, j):
                def TT(out, a, b_, op):
                    eng.tensor_tensor(out=out, in0=a, in1=b_, op=op)

                tA = pool.tile([P, 64], dt.bfloat16, name=f"tA{j}")
                tB = pool.tile([P, 64], dt.bfloat16, name=f"tB{j}")
                REL = pool.tile([P, RANK], dt.bfloat16, name=f"REL{j}")
                SS = pool.tile([P, RANK], dt.bfloat16, name=f"SS{j}")
                DD = pool.tile([P, RANK], dt.bfloat16, name=f"DD{j}")
                VV = pool.tile([P, RANK], dt.bfloat16, name=f"VV{j}")

                L0, L1 = Lg[:, j, 0:64], Lg[:, j, 64:128]
                W10, W11 = Wg[:, j, 128:192], Wg[:, j, 192:256]
                W20, W21 = Wg[:, j, 256:320], Wg[:, j, 320:384]
                CT0, CT1 = Tg[:, j, 0:64], Tg[:, j, 64:128]
                TMf = Tg[:, j, 128:256]
                TE0, TE1 = Tg[:, j, 256:320], Tg[:, j, 320:384]
                a, b_ = tA[:], tB[:]
                R0, R1 = REL[:, 0:64], REL[:, 64:128]
                Rf, W0f = REL[:], Wg[:, j, 0:128]
                S0, S1, Sf = SS[:, 0:64], SS[:, 64:128], SS[:]
                D0, D1, Df = DD[:, 0:64], DD[:, 64:128], DD[:]
                V0, V1 = VV[:, 0:64], VV[:, 64:128]

                # rel = W0 + L*W2 + CT*W1   (complex products; L*W2 first
                # because the W and L gathers land before T)
                TT(R0, L0, W20, mult)
                TT(b_, L1, W21, mult)
                TT(R0, R0, b_, sub)
                TT(R1, L0, W21, mult)
                TT(b_, L1, W20, mult)
                TT(R1, R1, b_, add)
                TT(Rf, Rf, W0f, add)
                TT(a, CT0, W10, mult)
                TT(b_, CT1, W11, mult)
                TT(a, a, b_, sub)
                TT(R0, R0, a, add)
                TT(a, CT0, W11, mult)
                TT(b_, CT1, W10, mult)
                TT(a, a, b_, add)
                TT(R1, R1, a, add)
                # S = rel + TM ; D = rel - TM
                TT(Sf, Rf, TMf, add)
                TT(Df, Rf, TMf, sub)
                # v = L*S + TE*conj(D)
                TT(a, L0, S0, mult)
                TT(b_, L1, S1, mult)
                TT(V0, a, b_, sub)
                TT(a, TE0, D0, mult)
                TT(b_, TE1, D1, mult)
                TT(a, a, b_, add)
                TT(V0, V0, a, add)
                TT(a, L0, S1, mult)
                TT(b_, L1, S0, mult)
                TT(V1, a, b_, add)
                TT(a, TE1, D0, mult)
                TT(b_, TE0, D1, mult)
                TT(a, a, b_, sub)
                TT(V1, V1, a, add)
                return VV

            def transp(VV, j):
                vt_ps = vtp.tile([P, P], dt.bfloat16, space="PSUM", tag="vtps")
                nc.tensor.transpose(out=vt_ps[:], in_=VV[:],
                                    identity=identb[:])
                vt = constp.tile([P, P], dt.bfloat16, name=f"vt{j}")
                nc.scalar.copy(out=vt[:], in_=vt_ps[:])
                return vt

            # heads 0-1 on DVE (transposed immediately); heads 2-3 on GpSimd,
            # their transposes emitted mid-stream so PE/ACT program order
            # doesn't block tiles 0-1 on the slower GpSimd heads.
            vts = [transp(head(nc.vector, ewa, 0), 0),
                   transp(head(nc.vector, ewa, 1), 1)]
            v2 = head(nc.gpsimd, ewb, 2)
            v3 = head(nc.gpsimd, ewb, 3)

            # ---- stream: per tile, 20 matmul chunks; casts 13:7 ACT:DVE;
            # output blocks DMA'd at OUT_EDGES boundaries.
            for j in range(TILES):
                if j == 2:
                    vts.append(transp(v2, 2))
                    vts.append(transp(v3, 3))
                vt = vts[j]
                osb = osbp.tile([P, NCOL], dt.bfloat16, tag="osb")
                edge = 1
                for c0 in range(0, NCOL, CHUNK):
                    k = c0 // CHUNK
                    mm = psm.tile([P, CHUNK], dt.float32, space="PSUM", tag="mm")
                    nc.tensor.matmul(out=mm[:], lhsT=vt[:],
                                     rhs=e0t[:, c0:c0 + CHUNK],
                                     start=True, stop=True)
                    if CAST_PAT[k] == "a":
                        nc.scalar.copy(out=osb[:, c0:c0 + CHUNK], in_=mm[:])
                    else:
                        nc.vector.tensor_copy(out=osb[:, c0:c0 + CHUNK],
                                              in_=mm[:])
                    if c0 + CHUNK >= OUT_EDGES[edge]:
                        o0, o1 = OUT_EDGES[edge - 1], OUT_EDGES[edge]
                        nc.sync.dma_start(OUT[j * P:(j + 1) * P, o0:o1],
                                          osb[:, o0:o1])
                        edge += 1

    nc.compile()
    return nc


def _prep_inputs(inputs):
    bf = ml_dtypes.bfloat16
    x = np.asarray(inputs["x"])
    E0 = np.asarray(inputs["E0"], dtype=np.float32)
    E1 = np.asarray(inputs["E1"], dtype=np.float32)
    E2 = np.asarray(inputs["E2"], dtype=np.float32)
    E3 = np.asarray(inputs["E3"], dtype=np.float32)
    E4 = np.asarray(inputs["E4"], dtype=np.float32)
    E5 = np.asarray(inputs["E5"], dtype=np.float32)
    E6 = np.asarray(inputs["E6"], dtype=np.float32)
    rule_C = np.asarray(inputs["rule_C"], dtype=np.float32)
    rule_S = np.asarray(inputs["rule_S"], dtype=np.float32)
    hr = np.asarray(inputs["has_rules"])[:, None]

    # per-relation folded tables (see module docstring)
    R0, R1 = E1[:, :64], E1[:, 64:]
    RC0, RC1 = rule_C[:, :64], rule_C[:, 64:]
    r2 = R0 * R0 + R1 * R1
    rs = rule_S[:, None]
    W0 = np.concatenate([np.where(hr, R0 - rs * r2, R0), R1], axis=1)
    W1 = np.concatenate(
        [np.where(hr, R0 * (1 + RC0) + R1 * RC1, R0),
         np.where(hr, R0 * RC1 - R1 * (1 + RC0), -R1)], axis=1)
    W2 = np.concatenate(
        [np.where(hr, 0.0, R0 + r2), np.where(hr, 0.0, -R1)], axis=1)
    wcat = np.ascontiguousarray(
        np.concatenate([W0, W1, W2], axis=1)).astype(bf)

    tb = np.arange(NTIME) // CYCLE
    tcat = np.ascontiguousarray(
        np.concatenate([E4, E2 + E5[tb], E3 + E6[tb]], axis=1)).astype(bf)

    e0b = np.ascontiguousarray(E0).astype(bf)
    e0t = np.ascontiguousarray(E0.T).astype(bf)
    e0t_shards = [np.ascontiguousarray(e0t[:, s * NCOL:(s + 1) * NCOL])
                  for s in range(SE)]

    idx = np.empty((B, 3), np.int32)
    idx[:, 0] = x[:, 0]
    idx[:, 1] = x[:, 1]
    idx[:, 2] = x[:, 3]

    in_maps = []
    for c in range(NCORES):
        sb = c // SE
        blk = idx[sb * 512:(sb + 1) * 512]          # [512, 3]
        # IDX[p, k, j] = index k of example sb*512 + j*128 + p
        idx3 = np.ascontiguousarray(
            blk.reshape(TILES, P, 3).transpose(1, 2, 0))
        in_maps.append({
            "IDX": idx3, "E0B": e0b, "WCAT": wcat, "TCAT": tcat,
            "E0T": e0t_shards[c % SE],
        })
    return in_maps


def kernel(**inputs):
    from concourse.bass_utils import run_bass_kernel_spmd

    if "nc" not in _CACHE:
        _CACHE["nc"] = _build()
    nc = _CACHE["nc"]

    in_maps = _prep_inputs(inputs)
    res = run_bass_kernel_spmd(nc, in_maps, core_ids=list(range(NCORES)),
                               trace=TRACE)
    _CACHE["last_result"] = res
    out = np.empty((B, NENT), np.float32)
    for c in range(NCORES):
        sb, se = c // SE, c % SE
        out[sb * 512:(sb + 1) * 512,
            se * NCOL:(se + 1) * NCOL] = res.results[c]["OUT"]
    return out


# revision 12
# speedup vs baseline: 1.0011x; 1.0011x over previous
"""Trainium2 Bass kernel for nn_CTRule (temporal KG scoring model).

Computes, for each of B=1024 queries (h, r, t):
  v = f(E0[h], E1[r], time tables, rule tables)   # [B, 128] elementwise algebra
  scores = v @ E0.T                               # [B, 40000]

Host-side table folding (index-independent, O(table-size) only):
  complex view u = u[:64] + i*u[64:].  The reference's rule/no-rule select
  collapses per relation r into three tables (hr = has_rules[r]):
    W0 = hr ? R - rule_S*|R|^2          : R
    W1 = hr ? conj(R)*(1 + rule_C)      : conj(R)
    W2 = hr ? 0                         : conj(R) + |R|^2
  so that  rel_ = W0[r] + CT*W1[r] + L*W2[r]   (complex products), and
    v = L*(rel_ + TM) + TE*conj(rel_ - TM)
  with TM = (E2 + E5[t//C])[t], TE = (E3 + E6[t//C])[t], CT = E4[t].

Distribution over 8 cores: 4-way model parallel over entities x 2-way data
parallel over the batch.  Core c = sb*4 + se handles batch rows
[sb*512, sb*512+512) (4 tiles of 128) x entity columns
[se*10000, se*10000+10000).  Per core:
  * 3 batched indirect gathers (SWDGE) pull W / L / T rows for all 4 tiles,
  * the bf16 elementwise head runs for tiles 0-1 on VectorE (2x_1P mode,
    two tiles per instruction) and tiles 2-3 on GpSimd in parallel,
  * vT transposes on TensorE (copied out of PSUM by ScalarE), then a
    500-column matmul stream against the bf16 E0T slice with PSUM->SBUF
    bf16 casts split 12:8 between ScalarE and VectorE,
  * [128, 2500] output blocks stream out on the sync-engine HWDGE ring.
No cross-core communication; the host reassembles the 8 blocks and casts f32.
"""

import numpy as np
import ml_dtypes

P = 128
B = 1024
RANK = 128
NENT = 40000
NREL = 230
NTIME = 365
CYCLE = 120
NCORES = 8
SE = 4                   # entity shards
SB = 2                   # batch shards
NCOL = NENT // SE        # entity columns per core = 10000
TILES = B // SB // P     # batch tiles per core = 4
CHUNK = 500              # matmul / PSUM chunk columns
OUTCH = 2500             # output DMA chunk columns
ECH = 2500               # E0T load chunk columns

W_W = 3 * RANK           # [W0 | W1 | W2] = 384
T_W = 3 * RANK           # [CT | TM | TE] = 384

# cast-engine schedule per tile (20 chunks): a=scalar(ACT), d=vector(DVE).
# ACT-heavier early (DVE is still finishing the tile-0/1 heads), even later.
def _pat(nd):
    s = ["a"] * 20
    for i in range(nd):
        s[(i * 20 // nd) + 1] = "d"
    return "".join(s)


CAST_PATS = [_pat(7), _pat(8), _pat(9), _pat(10)]
# output DMA group boundaries (columns) per tile: small first group so the
# out stream starts early
OUT_EDGES = [0, 1000, 4000, 7000, 10000]
# PE warm-up: junk matmuls keep the tensor engine's activity monitor hot so
# the real stream starts at full clock (HAM throttles up after ~3.4us of
# activity and back down when idle)
N_DUMMY = 90

TRACE = False            # set by test harness for profiling runs
_CACHE = {}


def _build():
    import concourse.bass as bass
    import concourse.mybir as mybir
    import concourse.tile as tile
    from concourse import bacc
    from concourse.masks import make_identity

    dt = mybir.dt
    mult = mybir.AluOpType.mult
    add = mybir.AluOpType.add
    sub = mybir.AluOpType.subtract

    nc = bacc.Bacc("TRN2", target_bir_lowering=False, debug=False,
                   num_devices=NCORES)

    IDX = nc.dram_tensor("IDX", [P, 3, TILES], dt.int32, kind="ExternalInput").ap()
    E0B = nc.dram_tensor("E0B", [NENT, RANK], dt.bfloat16, kind="ExternalInput").ap()
    WCAT = nc.dram_tensor("WCAT", [NREL, W_W], dt.bfloat16, kind="ExternalInput").ap()
    TCAT = nc.dram_tensor("TCAT", [NTIME, T_W], dt.bfloat16, kind="ExternalInput").ap()
    E0T = nc.dram_tensor("E0T", [RANK, NCOL], dt.bfloat16, kind="ExternalInput").ap()
    OUT = nc.dram_tensor("OUT", [TILES * P, NCOL], dt.bfloat16,
                         kind="ExternalOutput").ap()

    with tile.TileContext(nc) as tc:
        with (
            tc.tile_pool(name="const", bufs=1) as constp,
            tc.tile_pool(name="gath", bufs=1) as gp,
            tc.tile_pool(name="ewa", bufs=1) as ewa,
            tc.tile_pool(name="ewb", bufs=1) as ewb,
            tc.tile_pool(name="vtp", bufs=2, space="PSUM") as vtp,
            tc.tile_pool(name="psm", bufs=6, space="PSUM") as psm,
            tc.tile_pool(name="osbp", bufs=2) as osbp,
        ):
            # ---- index load + E0T stream on the sync HWDGE ring (fast path,
            # no gpsimd involvement); OUT blocks join the same ring later.
            idxt = gp.tile([P, 3, TILES], dt.int32)
            nc.sync.dma_start(idxt[:], IDX[:])
            e0t = constp.tile([RANK, NCOL], dt.bfloat16)
            for c0 in range(0, NCOL, ECH):
                nc.sync.dma_start(e0t[:, c0:c0 + ECH], E0T[:, c0:c0 + ECH])

            # identity first: it has no input deps, and the PE warm-up stream
            # (below) needs it as early as possible
            identb = constp.tile([P, P], dt.bfloat16)
            make_identity(nc, identb[:])

            # ---- PE warm-up: junk [128,128] matmuls from ~4us until the
            # first real transpose arrives, holding the PE clock at 2.4GHz
            for w in range(N_DUMMY):
                warm = vtp.tile([P, P], dt.float32, space="PSUM", tag="vtps")
                nc.tensor.matmul(out=warm[:], lhsT=identb[:], rhs=identb[:],
                                 start=True, stop=True)

            # ---- per-tile indirect gathers on SWDGE (the HW consumes exactly
            # one offset per partition per DMA), in tile order so tile 0's
            # head starts as early as possible
            Lg = gp.tile([P, TILES, RANK], dt.bfloat16)
            Wg = gp.tile([P, TILES, W_W], dt.bfloat16)
            Tg = gp.tile([P, TILES, T_W], dt.bfloat16)
            for j in range(TILES):
                for dst, src, col in ((Wg, WCAT, 1), (Lg, E0B, 0),
                                      (Tg, TCAT, 2)):
                    nc.gpsimd.indirect_dma_start(
                        out=dst[:, j, :], out_offset=None, in_=src[:],
                        in_offset=bass.IndirectOffsetOnAxis(
                            ap=idxt[:, col, j:j + 1], axis=0))

            # ---- elementwise head, all bf16, one tile per pass:
            # tiles 0-1 on VectorE (fast start), tiles 2-3 on GpSimd.
            def head(eng, pool, j):
                def TT(out, a, b_, op):
                    eng.tensor_tensor(out=out, in0=a, in1=b_, op=op)

                tA = pool.tile([P, 64], dt.bfloat16, name=f"tA{j}")
                tB = pool.tile([P, 64], dt.bfloat16, name=f"tB{j}")
                REL = pool.tile([P, RANK], dt.bfloat16, name=f"REL{j}")
                SS = pool.tile([P, RANK], dt.bfloat16, name=f"SS{j}")
                DD = pool.tile([P, RANK], dt.bfloat16, name=f"DD{j}")
                VV = pool.tile([P, RANK], dt.bfloat16, name=f"VV{j}")

                L0, L1 = Lg[:, j, 0:64], Lg[:, j, 64:128]
                W10, W11 = Wg[:, j, 128:192], Wg[:, j, 192:256]
                W20, W21 = Wg[:, j, 256:320], Wg[:, j, 320:384]
                CT0, CT1 = Tg[:, j, 0:64], Tg[:, j, 64:128]
                TMf = Tg[:, j, 128:256]
                TE0, TE1 = Tg[:, j, 256:320], Tg[:, j, 320:384]
                a, b_ = tA[:], tB[:]
                R0, R1 = REL[:, 0:64], REL[:, 64:128]
                Rf, W0f = REL[:], Wg[:, j, 0:128]
                S0, S1, Sf = SS[:, 0:64], SS[:, 64:128], SS[:]
                D0, D1, Df = DD[:, 0:64], DD[:, 64:128], DD[:]
                V0, V1 = VV[:, 0:64], VV[:, 64:128]

                # rel = W0 + L*W2 + CT*W1   (complex products; L*W2 first
                # because the W and L gathers land before T)
                TT(R0, L0, W20, mult)
                TT(b_, L1, W21, mult)
                TT(R0, R0, b_, sub)
                TT(R1, L0, W21, mult)
                TT(b_, L1, W20, mult)
                TT(R1, R1, b_, add)
                TT(Rf, Rf, W0f, add)
                TT(a, CT0, W10, mult)
                TT(b_, CT1, W11, mult)
                TT(a, a, b_, sub)
                TT(R0, R0, a, add)
                TT(a, CT0, W11, mult)
                TT(b_, CT1, W10, mult)
                TT(a, a, b_, add)
                TT(R1, R1, a, add)
                # S = rel + TM ; D = rel - TM
                TT(Sf, Rf, TMf, add)
                TT(Df, Rf, TMf, sub)
                # v = L*S + TE*conj(D)
                TT(a, L0, S0, mult)
                TT(b_, L1, S1, mult)
                TT(V0, a, b_, sub)
                TT(a, TE0, D0, mult)
                TT(b_, TE1, D1, mult)
                TT(a, a, b_, add)
                TT(V0, V0, a, add)
                TT(a, L0, S1, mult)
                TT(b_, L1, S0, mult)
                TT(V1, a, b_, add)
                TT(a, TE1, D0, mult)
                TT(b_, TE0, D1, mult)
                TT(a, a, b_, sub)
                TT(V1, V1, a, add)
                return VV

            def transp(VV, j):
                vt_ps = vtp.tile([P, P], dt.bfloat16, space="PSUM", tag="vtps")
                nc.tensor.transpose(out=vt_ps[:], in_=VV[:],
                                    identity=identb[:])
                vt = constp.tile([P, P], dt.bfloat16, name=f"vt{j}")
                nc.scalar.copy(out=vt[:], in_=vt_ps[:])
                return vt

            # heads 0-1 on DVE (transposed immediately); heads 2-3 on GpSimd,
            # their transposes emitted mid-stream so PE/ACT program order
            # doesn't block tiles 0-1 on the slower GpSimd heads.
            vts = [transp(head(nc.vector, ewa, 0), 0),
                   transp(head(nc.vector, ewa, 1), 1)]
            v2 = head(nc.gpsimd, ewb, 2)
            v3 = head(nc.gpsimd, ewb, 3)

            # ---- stream: per tile, 20 matmul chunks; casts split ACT/DVE;
            # output blocks DMA'd at OUT_EDGES boundaries.
            for j in range(TILES):
                if j == 2:
                    vts.append(transp(v2, 2))
                    vts.append(transp(v3, 3))
                vt = vts[j]
                osb = osbp.tile([P, NCOL], dt.bfloat16, tag="osb")
                edge = 1
                for c0 in range(0, NCOL, CHUNK):
                    k = c0 // CHUNK
                    mm = psm.tile([P, CHUNK], dt.float32, space="PSUM", tag="mm")
                    nc.tensor.matmul(out=mm[:], lhsT=vt[:],
                                     rhs=e0t[:, c0:c0 + CHUNK],
                                     start=True, stop=True)
                    if CAST_PATS[j][k] == "a":
                        nc.scalar.copy(out=osb[:, c0:c0 + CHUNK], in_=mm[:])
                    else:
                        nc.vector.tensor_copy(out=osb[:, c0:c0 + CHUNK],
                                              in_=mm[:])
                    if c0 + CHUNK >= OUT_EDGES[edge]:
                        o0, o1 = OUT_EDGES[edge - 1], OUT_EDGES[edge]
                        nc.sync.dma_start(OUT[j * P:(j + 1) * P, o0:o1],
                                          osb[:, o0:o1])
                        edge += 1

    nc.compile()
    return nc


def _prep_inputs(inputs):
    bf = ml_dtypes.bfloat16
    x = np.asarray(inputs["x"])
    E0 = np.asarray(inputs["E0"], dtype=np.float32)
    E1 = np.asarray(inputs["E1"], dtype=np.float32)
    E2 = np.asarray(inputs["E2"], dtype=np.float32)
    E3 = np.asarray(inputs["E3"], dtype=np.float32)
    E4 = np.asarray(inputs["E4"], dtype=np.float32)
    E5 = np.asarray(inputs["E5"], dtype=np.float32)
    E6 = np.asarray(inputs["E6"], dtype=np.float32)
    rule_C = np.asarray(inputs["rule_C"], dtype=np.float32)
    rule_S = np.asarray(inputs["rule_S"], dtype=np.float32)
    hr = np.asarray(inputs["has_rules"])[:, None]

    # per-relation folded tables (see module docstring)
    R0, R1 = E1[:, :64], E1[:, 64:]
    RC0, RC1 = rule_C[:, :64], rule_C[:, 64:]
    r2 = R0 * R0 + R1 * R1
    rs = rule_S[:, None]
    W0 = np.concatenate([np.where(hr, R0 - rs * r2, R0), R1], axis=1)
    W1 = np.concatenate(
        [np.where(hr, R0 * (1 + RC0) + R1 * RC1, R0),
         np.where(hr, R0 * RC1 - R1 * (1 + RC0), -R1)], axis=1)
    W2 = np.concatenate(
        [np.where(hr, 0.0, R0 + r2), np.where(hr, 0.0, -R1)], axis=1)
    wcat = np.ascontiguousarray(
        np.concatenate([W0, W1, W2], axis=1)).astype(bf)

    tb = np.arange(NTIME) // CYCLE
    tcat = np.ascontiguousarray(
        np.concatenate([E4, E2 + E5[tb], E3 + E6[tb]], axis=1)).astype(bf)

    e0b = np.ascontiguousarray(E0).astype(bf)
    e0t = np.ascontiguousarray(E0.T).astype(bf)
    e0t_shards = [np.ascontiguousarray(e0t[:, s * NCOL:(s + 1) * NCOL])
                  for s in range(SE)]

    idx = np.empty((B, 3), np.int32)
    idx[:, 0] = x[:, 0]
    idx[:, 1] = x[:, 1]
    idx[:, 2] = x[:, 3]

    in_maps = []
    for c in range(NCORES):
        sb = c // SE
        blk = idx[sb * 512:(sb + 1) * 512]          # [512, 3]
        # IDX[p, k, j] = index k of example sb*512 + j*128 + p
        idx3 = np.ascontiguousarray(
            blk.reshape(TILES, P, 3).transpose(1, 2, 0))
        in_maps.append({
            "IDX": idx3, "E0B": e0b, "WCAT": wcat, "TCAT": tcat,
            "E0T": e0t_shards[c % SE],
        })
    return in_maps


def kernel(**inputs):
    from concourse.bass_utils import run_bass_kernel_spmd

    if "nc" not in _CACHE:
        _CACHE["nc"] = _build()
    nc = _CACHE["nc"]

    in_maps = _prep_inputs(inputs)
    res = run_bass_kernel_spmd(nc, in_maps, core_ids=list(range(NCORES)),
                               trace=TRACE)
    _CACHE["last_result"] = res
    out = np.empty((B, NENT), np.float32)
    for c in range(NCORES):
        sb, se = c // SE, c % SE
        out[sb * 512:(sb + 1) * 512,
            se * NCOL:(se + 1) * NCOL] = res.results[c]["OUT"]
    return out
